# revision 1
# baseline (speedup 1.0000x reference)
"""MoE feed-forward (8 experts, top-2) on 8 TRN2 NeuronCores, expert-parallel.

Strategy: core c holds expert c's weights. Tokens are sharded by position
(1024/core). Each core computes fp32 gating + top-2 for its tokens, assigns
per-expert slots via triangular-matmul prefix sums, scatters bf16 token rows
into an [E, C, D] send buffer with indirect DMA, AllToAll-dispatches them,
runs the expert MLP in bf16 (fp32 accumulate), AllToAll-returns bf16 results,
then gathers its tokens' two expert outputs and combines with the renormalized
routing weights.

Engine plan: PE matmuls + gating transposes; ACT silu/sigmoid/weight-casts;
sync HWDGE queue owns the 32MB weight stream + compute-phase DMA-transpose
loads; gpsimd (SWDGE, separate DMA-semaphore lanes) carries phase-A loads,
indirect scatters/gathers and collective triggers; DVE does the small vector
work and psum->sbuf bias adds. The 16 dispatch scatters write through
per-scatter DRAM aliases so Tile's same-tensor WAW tracking doesn't chain
them; the collective's dependency on them is declared explicitly.
"""
import numpy as np

import concourse.bass as bass
import concourse.mybir as mybir
import concourse.tile as tile
from concourse import bacc
from concourse.bass import IndirectOffsetOnAxis
from concourse.bass_utils import run_bass_kernel_spmd
from concourse.masks import make_identity, make_upper_triangular

D_MODEL, HIDDEN, N_EXPERTS, TOP_K = 1024, 4096, 8, 2
N_CORES = 8
P = 128
T = 8192
T_LOC = T // N_CORES            # 1024 tokens per core
N_TOK_TILES = T_LOC // P        # 8
D_BLKS = D_MODEL // P           # 8
H_BLKS = HIDDEN // P            # 32
N_CT = 512                      # token tile in expert-compute phase

FP32 = mybir.dt.float32
BF16 = mybir.dt.bfloat16
I32 = mybir.dt.int32
U32 = mybir.dt.uint32
AF = mybir.ActivationFunctionType
ALU = mybir.AluOpType

RG = [list(range(N_CORES))]


def _dram_alias(nc, base_handle, name):
    """A DRAM tensor handle aliasing base_handle's memory. Distinct names keep
    Tile's conservative same-tensor WAW tracking from serializing writers that
    are known (by construction) to touch disjoint rows."""
    mls = nc._tensor(name, list(base_handle.shape), base_handle.dtype,
                     kind="Internal", type="DRAM")
    base_mloc = nc.lookup_mloc(base_handle)
    mloc = mls.memorylocations[0]
    mloc.allocated = base_mloc.allocated
    mloc.addr = base_mloc.addr
    return bass.DRamTensorHandle(name, list(base_handle.shape),
                                 base_handle.dtype)


def _body(tc, C, S_cap, x_loc, gate_w, gate_b_rep, iota8_rep, w1_loc, b1_loc, w2_loc,
          b2_rep, riota_rep, out_loc):
    nc = tc.nc
    S = N_EXPERTS * C

    send_x_t = nc.dram_tensor("send_x_buf", [S, D_MODEL], BF16)
    send_x_aliases = [_dram_alias(nc, send_x_t, f"send_x_al{i}")
                      for i in range(N_TOK_TILES * TOP_K)]
    send_y_t = nc.dram_tensor("send_y_buf", [S, D_MODEL], BF16)
    send_y_aliases = [_dram_alias(nc, send_y_t, f"send_y_al{i}")
                      for i in range(S_cap // P)]

    with tc.tile_pool(name="dram", bufs=1, space="DRAM") as dram, \
         tc.tile_pool(name="persist", bufs=1) as persist:
        send_x = send_x_t.ap()
        recv_x = dram.tile([S, D_MODEL], BF16)
        send_y = send_y_t.ap()
        recv_y = dram.tile([S, D_MODEL], BF16)
        compact_x = dram.tile([S_cap, D_MODEL], BF16)
        cnt_send = dram.tile([N_CORES, 64], FP32)
        cnt_recv = dram.tile([N_CORES, 64], FP32)

        ident = persist.tile([P, P], FP32)
        make_identity(nc, ident)
        strictu = persist.tile([P, P], FP32)
        make_upper_triangular(nc, strictu, val=1.0, diag=False)
        ones_t = persist.tile([P, P], FP32)
        nc.gpsimd.memset(ones_t, 1.0)

        gb_sb = persist.tile([P, N_EXPERTS], FP32)
        nc.gpsimd.dma_start(gb_sb, gate_b_rep[:])
        iota_sb = persist.tile([P, N_EXPERTS], FP32)
        nc.gpsimd.dma_start(iota_sb, iota8_rep[:])
        gw_sb = persist.tile([P, D_BLKS, N_EXPERTS], FP32)
        nc.gpsimd.dma_start(gw_sb, gate_w[:].rearrange("(j p) e -> p j e", p=P))
        b1_sb = persist.tile([P, H_BLKS], FP32)
        nc.gpsimd.dma_start(b1_sb, b1_loc[:])
        b2r_sb = persist.tile([P, D_MODEL], FP32)
        nc.gpsimd.dma_start(b2r_sb, b2_rep[:])

        rows_sb = persist.tile([P, N_TOK_TILES, TOP_K], I32)
        wts_sb = persist.tile([P, N_TOK_TILES, TOP_K], FP32)
        sendmask = persist.tile([P, N_TOK_TILES * N_EXPERTS], FP32)

        w1_sb = persist.tile([P, D_BLKS, HIDDEN], BF16)
        w2_sb = persist.tile([P, H_BLKS, D_MODEL], BF16)

        # ---- expert weights: fp32 DMA on the sync queue (nothing else runs
        # there until phase C), cast to bf16 on ACT. h-major for w1 so the
        # first hidden blocks are ready as soon as compute starts. ----
        W_CHUNK = 1024
        with tc.tile_pool(name="wstage", bufs=3) as wstage, \
             tc.tile_pool(name="phA", bufs=2) as pA, \
             tc.tile_pool(name="phA8", bufs=N_TOK_TILES) as pA8, \
             tc.tile_pool(name="phA_psum", bufs=2, space="PSUM") as pAp:
            for h in range(HIDDEN // W_CHUNK):
                for j in range(D_BLKS):
                    wst = wstage.tile([P, W_CHUNK], FP32, tag="wst", name="wst")
                    nc.sync.dma_start(wst, w1_loc[j * P:(j + 1) * P,
                                                  h * W_CHUNK:(h + 1) * W_CHUNK])
                    nc.scalar.activation(
                        w1_sb[:, j, h * W_CHUNK:(h + 1) * W_CHUNK], wst, AF.Copy)
            for m in range(H_BLKS):
                wst = wstage.tile([P, W_CHUNK], FP32, tag="wst", name="wst")
                nc.sync.dma_start(wst, w2_loc[m * P:(m + 1) * P, :])
                nc.scalar.activation(w2_sb[:, m, :], wst, AF.Copy)

            # ---- phase A: gating + routing + dispatch scatter ----
            # issue every x load up front (own bufs) so no ACT wait can
            # block a later load issue on the in-order engine queue
            x_bf_all = pA8.tile([P, N_TOK_TILES, D_MODEL], BF16, bufs=1)
            x_tiles = []
            for i in range(N_TOK_TILES):
                x_sb = pA8.tile([P, D_MODEL], FP32, tag="x_sb", name="x_sb")
                nc.gpsimd.dma_start(x_sb, x_loc[i * P:(i + 1) * P, :])
                x_tiles.append(x_sb)

            eqs = []

            def emit_gating(i):
                x_sb = x_tiles[i]
                nc.vector.tensor_copy(x_bf_all[:, i, :], x_sb)

                xT = pA.tile([P, D_BLKS, P], FP32, tag="xT", name="xT")
                for j in range(D_BLKS):
                    tp = pAp.tile([P, P], FP32, tag="tp", name="tp")
                    nc.tensor.transpose(tp, x_sb[:, j * P:(j + 1) * P], ident)
                    nc.vector.tensor_copy(xT[:, j, :], tp)

                gps = pAp.tile([P, N_EXPERTS], FP32, tag="gps", name="gps")
                for j in range(D_BLKS):
                    nc.tensor.matmul(gps, lhsT=xT[:, j, :], rhs=gw_sb[:, j, :],
                                     start=(j == 0), stop=(j == D_BLKS - 1))
                logits = pA.tile([P, N_EXPERTS], FP32, tag="logits", name="logits")
                nc.vector.tensor_add(logits, gps, gb_sb)

                maxv = pA.tile([P, 8], FP32, tag="maxv", name="maxv")
                nc.vector.max(maxv, logits)
                maxi = pA.tile([P, 8], U32, tag="maxi", name="maxi")
                nc.vector.max_index(maxi, maxv, logits)

                d01 = pA.tile([P, 1], FP32, tag="d01", name="d01")
                nc.vector.tensor_sub(d01, maxv[:, 0:1], maxv[:, 1:2])
                # renormalized top-2: w0 = sigmoid(l0-l1), w1 = sigmoid(l1-l0)
                nc.scalar.activation(wts_sb[:, i, 0:1], d01, AF.Sigmoid)
                nc.scalar.activation(wts_sb[:, i, 1:2], d01, AF.Sigmoid,
                                     scale=-1.0)

                idxf = pA8.tile([P, TOP_K], FP32, tag="idxf", name="idxf")
                nc.vector.tensor_copy(idxf, maxi[:, 0:TOP_K])
                eq0 = pA8.tile([P, N_EXPERTS], FP32, tag="eq0", name="eq0")
                nc.vector.tensor_tensor(
                    eq0, idxf[:, 0:1].to_broadcast([P, N_EXPERTS]),
                    iota_sb, op=ALU.is_equal)
                eq1 = pA8.tile([P, N_EXPERTS], FP32, tag="eq1", name="eq1")
                nc.vector.tensor_tensor(
                    eq1, idxf[:, 1:2].to_broadcast([P, N_EXPERTS]),
                    iota_sb, op=ALU.is_equal)
                eqs.append((idxf, eq0, eq1))
                nc.vector.tensor_add(
                    sendmask[:, i * N_EXPERTS:(i + 1) * N_EXPERTS], eq0, eq1)

            offs = pA.tile([P, N_TOK_TILES, N_EXPERTS], FP32, tag="offs",
                           name="offs")
            csum_sb = pA.tile([P, N_TOK_TILES * N_EXPERTS], FP32,
                              tag="csum_sb", name="csum_sb")
            scatter_insts = []

            def emit_slots(i):
                idxf, eq0, eq1 = eqs[i]
                for k in range(TOP_K):
                    eqk = eq0 if k == 0 else eq1
                    prod = pA.tile([P, N_EXPERTS], FP32, tag="prod", name="prod")
                    nc.vector.tensor_mul(prod, offs[:, i, :], eqk)
                    slot = pA.tile([P, 1], FP32, tag="slot", name="slot")
                    nc.vector.reduce_sum(slot, prod, axis=mybir.AxisListType.X)
                    rowf = pA.tile([P, 1], FP32, tag="rowf", name="rowf")
                    nc.vector.tensor_scalar(rowf, idxf[:, k:k + 1], float(C),
                                            slot, op0=ALU.mult, op1=ALU.add)
                    nc.vector.tensor_copy(rows_sb[:, i, k:k + 1], rowf)
                    si = nc.gpsimd.indirect_dma_start(
                        out=send_x_aliases[i * TOP_K + k].ap(),
                        out_offset=IndirectOffsetOnAxis(
                            ap=rows_sb[:, i, k:k + 1], axis=0),
                        in_=x_bf_all[:, i, :],
                        in_offset=None,
                    )
                    scatter_insts.append(si)

            # two half-batches: tiles 0-3 reach their scatters while tiles
            # 4-7 are still gating
            HB = N_TOK_TILES // 2
            for b in range(2):
                base = b * HB
                for i in range(base, base + HB):
                    emit_gating(i)
                sl = slice(base * N_EXPERTS, (base + HB) * N_EXPERTS)
                pref_ps = pAp.tile([P, HB * N_EXPERTS], FP32, tag="pref",
                                   name="pref")
                nc.tensor.matmul(pref_ps, lhsT=strictu, rhs=sendmask[:, sl],
                                 start=True, stop=True)
                csum_ps = pAp.tile([P, HB * N_EXPERTS], FP32, tag="csum",
                                   name="csum")
                nc.tensor.matmul(csum_ps, lhsT=ones_t, rhs=sendmask[:, sl],
                                 start=True, stop=True)
                offs_flat = offs[:].rearrange("p a b -> p (a b)")
                nc.vector.tensor_copy(offs_flat[:, sl], pref_ps)
                nc.vector.tensor_copy(csum_sb[:, sl], csum_ps)
                if b == 1:
                    # carry inclusive colsum through tile HB-1 into batch 1
                    carry = csum_sb[:, (HB - 1) * N_EXPERTS:HB * N_EXPERTS]
                    nc.vector.tensor_add(offs[:, HB, :], offs[:, HB, :], carry)
                    cur0 = csum_sb[:, HB * N_EXPERTS:(HB + 1) * N_EXPERTS]
                    nc.vector.tensor_add(cur0, cur0, carry)
                for i in range(base + 1, base + HB):
                    prev = csum_sb[:, (i - 1) * N_EXPERTS:i * N_EXPERTS]
                    nc.vector.tensor_add(offs[:, i, :], offs[:, i, :], prev)
                    cur = csum_sb[:, i * N_EXPERTS:(i + 1) * N_EXPERTS]
                    nc.vector.tensor_add(cur, cur, prev)
                for i in range(base, base + HB):
                    emit_slots(i)

        # per-expert totals staged early; the tiny count A2A itself runs AFTER
        # the big dispatch A2A (a leading small collective pays the full
        # peer-skew barrier and delays the big one ~40us)
        nc.gpsimd.dma_start(cnt_send[:, 0:1],
                            csum_sb[0:1, (N_TOK_TILES - 1) * N_EXPERTS:
                                    N_TOK_TILES * N_EXPERTS])

        # ---- dispatch all-to-all (depends on every aliased scatter) ----
        cc1 = nc.gpsimd.collective_compute(
            "AllToAll", ALU.bypass, replica_groups=RG,
            ins=[send_x[:].opt()], outs=[recv_x[:].opt()])
        for si in scatter_insts:
            bass._add_dep_helper(cc1.ins, si.ins, sync=True,
                                 reason="a2a after aliased scatters")
        cc_cnt = nc.gpsimd.collective_compute(
            "AllToAll", ALU.bypass, replica_groups=RG,
            ins=[cnt_send[:].opt()], outs=[cnt_recv[:].opt()])

        # ---- phase C: compact the padded recv slots, then expert MLP ----
        NCOL = S_cap // P
        with tc.tile_pool(name="phC", bufs=2) as pC, \
             tc.tile_pool(name="phC_psum", bufs=2, space="PSUM") as pCp:
            # counts -> on-device compact gather table:
            #   gidx[r] = src(r)*C + (r - cumexcl(src(r)))  for r in [0, S_cap)
            cnt_row = pC.tile([1, N_CORES], FP32, tag="cnt_row", name="cnt_row")
            nc.scalar.dma_start(cnt_row, cnt_recv[:, 0:1])
            cum_row = pC.tile([1, N_CORES], FP32, tag="cum_row", name="cum_row")
            nc.vector.tensor_copy(cum_row, cnt_row)
            for s in range(1, N_CORES):
                nc.vector.tensor_add(cum_row[:, s:s + 1], cum_row[:, s:s + 1],
                                     cum_row[:, s - 1:s])
            bc1 = pCp.tile([P, N_CORES], FP32, tag="bc1", name="bc1", bufs=1)
            nc.tensor.matmul(bc1, lhsT=ones_t[0:1, :], rhs=cnt_row[:],
                             start=True, stop=True)
            bc2 = pCp.tile([P, N_CORES], FP32, tag="bc2", name="bc2", bufs=1)
            nc.tensor.matmul(bc2, lhsT=ones_t[0:1, :], rhs=cum_row[:],
                             start=True, stop=True)
            cnt_bc = pC.tile([P, N_CORES], FP32, tag="cnt_bc", name="cnt_bc")
            nc.vector.tensor_copy(cnt_bc, bc1)
            cum_bc = pC.tile([P, N_CORES], FP32, tag="cum_bc", name="cum_bc")
            nc.vector.tensor_copy(cum_bc, bc2)

            riota_f = pC.tile([P, NCOL], FP32, tag="riota_f", name="riota_f")
            nc.scalar.dma_start(riota_f, riota_rep[:])
            # fused mask build: one [P, NCOL, 8] outer-compare instead of a
            # 32-op chained loop (the table build is latency-critical)
            msk3 = pC.tile([P, NCOL, N_CORES], FP32, tag="msk3", name="msk3")
            nc.vector.tensor_tensor(
                msk3, riota_f[:, :, None].to_broadcast([P, NCOL, N_CORES]),
                cum_bc[:, None, :].to_broadcast([P, NCOL, N_CORES]),
                op=ALU.is_ge)
            s_of = pC.tile([P, NCOL], FP32, tag="s_of", name="s_of")
            nc.vector.reduce_sum(s_of, msk3[:], axis=mybir.AxisListType.X)
            wmsk3 = pC.tile([P, NCOL, N_CORES], FP32, tag="wmsk3", name="wmsk3")
            nc.vector.tensor_tensor(
                wmsk3, msk3[:],
                cnt_bc[:, None, :].to_broadcast([P, NCOL, N_CORES]),
                op=ALU.mult)
            cume = pC.tile([P, NCOL], FP32, tag="cume", name="cume")
            nc.vector.reduce_sum(cume, wmsk3[:], axis=mybir.AxisListType.X)
            gidx_f = pC.tile([P, NCOL], FP32, tag="gidx_f", name="gidx_f")
            nc.vector.tensor_scalar(gidx_f, s_of, float(C), None, op0=ALU.mult)
            nc.vector.tensor_add(gidx_f, gidx_f, riota_f)
            nc.vector.tensor_sub(gidx_f, gidx_f, cume)
            gidx = pC.tile([P, NCOL], I32, tag="gidx", name="gidx")
            nc.vector.tensor_copy(gidx, gidx_f)

            # compact bounce: recv_x -> compact_x (tail rows OOB-skipped)
            for col in range(NCOL):
                xg = pC.tile([P, D_MODEL], BF16, tag="xg", name="xg")
                nc.gpsimd.indirect_dma_start(
                    out=xg, out_offset=None, in_=recv_x[:],
                    in_offset=IndirectOffsetOnAxis(ap=gidx[:, col:col + 1],
                                                   axis=0),
                    bounds_check=S - 1, oob_is_err=False)
                nc.sync.dma_start(compact_x[col * P:(col + 1) * P, :], xg)

            # expert MLP over S_cap compacted slots (mixed 512/256 tiles)
            ret_scatters = []
            ctiles = []
            off = 0
            while off < S_cap:
                nt = min(N_CT, S_cap - off)
                ctiles.append((off, nt))
                off += nt
            for (r0, NT) in ctiles:
                xrT = pC.tile([P, D_BLKS, N_CT], BF16, tag="xrT", name="xrT", bufs=3)
                for j in range(D_BLKS):
                    nc.sync.dma_start(
                        xrT[:, j, :NT],
                        compact_x[r0:r0 + NT, j * P:(j + 1) * P],
                        transpose=True)
                hT = pC.tile([P, H_BLKS, N_CT], BF16, tag="hT", name="hT",
                             bufs=1)
                for m in range(H_BLKS):
                    ps1 = pCp.tile([P, N_CT], FP32, tag="ps1", name="ps1", bufs=3)
                    nc.tensor.matmul(ps1[:, :NT], lhsT=w1_sb[:, 0, m * P:(m + 1) * P],
                                     rhs=xrT[:, 0, :NT], start=True, stop=False)
                    for j in range(1, D_BLKS):
                        nc.tensor.matmul(ps1[:, :NT],
                                         lhsT=w1_sb[:, j, m * P:(m + 1) * P],
                                         rhs=xrT[:, j, :NT],
                                         start=False, stop=(j == D_BLKS - 1))
                    nc.scalar.activation(hT[:, m, :NT], ps1[:, :NT], AF.Silu,
                                         bias=b1_sb[:, m:m + 1])
                for t in range(NT // P):
                    col = (r0 + t * P) // P
                    y_tm = pC.tile([P, D_MODEL], BF16, tag="y_tm", name="y_tm", bufs=3)
                    for nh in range(2):
                        ps2 = pCp.tile([P, 512], FP32, tag="ps2", name="ps2")
                        for m in range(H_BLKS):
                            nc.tensor.matmul(
                                ps2, lhsT=hT[:, m, t * P:(t + 1) * P],
                                rhs=w2_sb[:, m, nh * 512:(nh + 1) * 512],
                                start=(m == 0), stop=(m == H_BLKS - 1))
                        nc.vector.tensor_add(y_tm[:, nh * 512:(nh + 1) * 512],
                                             ps2, b2r_sb[:, nh * 512:(nh + 1) * 512])
                    si = nc.gpsimd.indirect_dma_start(
                        out=send_y_aliases[col].ap(),
                        out_offset=IndirectOffsetOnAxis(ap=gidx[:, col:col + 1],
                                                        axis=0),
                        in_=y_tm[:],
                        in_offset=None,
                        bounds_check=S - 1, oob_is_err=False)
                    ret_scatters.append(si)

        # ---- return all-to-all (depends on every aliased return scatter) ----
        cc2 = nc.gpsimd.collective_compute(
            "AllToAll", ALU.bypass, replica_groups=RG,
            ins=[send_y[:].opt()], outs=[recv_y[:].opt()])
        for si in ret_scatters:
            bass._add_dep_helper(cc2.ins, si.ins, sync=True,
                                 reason="return a2a after aliased scatters")

        # ---- phase E: gather + weighted combine ----
        with tc.tile_pool(name="phE", bufs=2) as pE:
            for i in range(N_TOK_TILES):
                g0 = pE.tile([P, D_MODEL], BF16, tag="g0", name="g0")
                nc.gpsimd.indirect_dma_start(
                    out=g0, out_offset=None, in_=recv_y[:],
                    in_offset=IndirectOffsetOnAxis(ap=rows_sb[:, i, 0:1], axis=0))
                g1 = pE.tile([P, D_MODEL], BF16, tag="g1", name="g1")
                nc.gpsimd.indirect_dma_start(
                    out=g1, out_offset=None, in_=recv_y[:],
                    in_offset=IndirectOffsetOnAxis(ap=rows_sb[:, i, 1:2], axis=0))
                t0 = pE.tile([P, D_MODEL], FP32, tag="t0", name="t0")
                nc.vector.tensor_scalar_mul(t0, g0, wts_sb[:, i, 0:1])
                t1 = pE.tile([P, D_MODEL], FP32, tag="t1", name="t1")
                nc.vector.tensor_scalar_mul(t1, g1, wts_sb[:, i, 1:2])
                out_t = pE.tile([P, D_MODEL], FP32, tag="out_t", name="out_t")
                nc.vector.tensor_add(out_t, t0, t1)
                nc.scalar.dma_start(out_loc[i * P:(i + 1) * P, :], out_t)


def build_kernel(C, S_cap):
    nc = bacc.Bacc("TRN2", target_bir_lowering=False, debug=False,
                   num_devices=N_CORES)
    args = dict(
        x_loc=nc.dram_tensor("x_loc", [T_LOC, D_MODEL], FP32, kind="ExternalInput"),
        gate_w=nc.dram_tensor("gate_w", [D_MODEL, N_EXPERTS], FP32, kind="ExternalInput"),
        gate_b_rep=nc.dram_tensor("gate_b_rep", [P, N_EXPERTS], FP32, kind="ExternalInput"),
        iota8_rep=nc.dram_tensor("iota8_rep", [P, N_EXPERTS], FP32, kind="ExternalInput"),
        w1_loc=nc.dram_tensor("w1_loc", [D_MODEL, HIDDEN], FP32, kind="ExternalInput"),
        b1_loc=nc.dram_tensor("b1_loc", [P, H_BLKS], FP32, kind="ExternalInput"),
        w2_loc=nc.dram_tensor("w2_loc", [HIDDEN, D_MODEL], FP32, kind="ExternalInput"),
        b2_rep=nc.dram_tensor("b2_rep", [P, D_MODEL], FP32, kind="ExternalInput"),
        riota_rep=nc.dram_tensor("riota_rep", [P, S_cap // P], FP32,
                                 kind="ExternalInput"),
        out_loc=nc.dram_tensor("out_loc", [T_LOC, D_MODEL], FP32, kind="ExternalOutput"),
    )
    with tile.TileContext(nc) as tc:
        _body(tc, C, S_cap, **{k: v.ap() for k, v in args.items()})
    nc.compile()
    return nc


def _capacity(flat_x, gate_w, gate_b):
    """A2A chunk capacity C (max per (src, expert) count) and compact compute
    bound S_cap (max per-expert total), both from the actual input with an
    +8 margin against tiny fp reorder flips between host and device gating."""
    logits = flat_x @ gate_w + gate_b
    top2 = np.argsort(-logits, axis=1, kind="stable")[:, :TOP_K]
    blocks = top2.reshape(N_CORES, T_LOC, TOP_K)
    counts = np.stack([(blocks == e).sum(axis=(1, 2)) for e in range(N_EXPERTS)])
    C = ((int(counts.max()) + 8 + 63) // 64) * 64
    S_cap = ((int(counts.sum(axis=1).max()) + 8 + 127) // 128) * 128
    return C, S_cap


_CACHE = {}


def kernel(x, gate_w, gate_b, w1, b1, w2, b2, _trace=False):
    x = np.ascontiguousarray(np.asarray(x, dtype=np.float32))
    gate_w = np.ascontiguousarray(np.asarray(gate_w, dtype=np.float32))
    gate_b = np.ascontiguousarray(np.asarray(gate_b, dtype=np.float32))
    w1 = np.ascontiguousarray(np.asarray(w1, dtype=np.float32))
    b1 = np.ascontiguousarray(np.asarray(b1, dtype=np.float32))
    w2 = np.ascontiguousarray(np.asarray(w2, dtype=np.float32))
    b2 = np.ascontiguousarray(np.asarray(b2, dtype=np.float32))

    orig_shape = x.shape
    flat_x = x.reshape(-1, D_MODEL)
    C, S_cap = _capacity(flat_x, gate_w, gate_b)

    if (C, S_cap) not in _CACHE:
        _CACHE[(C, S_cap)] = build_kernel(C, S_cap)
    nc = _CACHE[(C, S_cap)]

    iota8 = np.tile(np.arange(N_EXPERTS, dtype=np.float32), (P, 1))
    ncol = S_cap // P
    riota_cols = (np.arange(P, dtype=np.float32)[:, None]
                  + P * np.arange(ncol, dtype=np.float32)[None, :])
    riota_cols = np.ascontiguousarray(riota_cols)
    gb_rep = np.tile(gate_b, (P, 1))
    in_maps = []
    for c in range(N_CORES):
        in_maps.append({
            "x_loc": flat_x[c * T_LOC:(c + 1) * T_LOC],
            "gate_w": gate_w,
            "gate_b_rep": gb_rep,
            "iota8_rep": iota8,
            "w1_loc": w1[c],
            "b1_loc": np.ascontiguousarray(b1[c].reshape(H_BLKS, P).T),
            "w2_loc": w2[c],
            "b2_rep": np.tile(b2[c], (P, 1)),
            "riota_rep": riota_cols,
        })

    res = run_bass_kernel_spmd(nc, in_maps, core_ids=list(range(N_CORES)),
                               trace=_trace)
    out = np.concatenate([res.results[c]["out_loc"] for c in range(N_CORES)],
                         axis=0)
    if _trace:
        kernel.last_results = res
    return out.reshape(orig_shape)

